# revision 18
# baseline (speedup 1.0000x reference)
"""Bass/Tile attention kernel for trn2, data-parallel over batch on 8 cores.

Computes, per batch b:
    q = x_to @ Wq + bq ; k = x_from @ Wk + bk ; v = x_from @ Wv + bv
    out = softmax(q k^T / sqrt(H)) @ v

Layout strategy (per core, 2 batches):
  - x transposed on PE (identity matmul) into xT [d, seq] tiles.
  - kT [h, k] = Wk^T x_from^T, qT [h, q] = Wq^T x_to^T  (ACT eviction adds bias
    per-partition), v [k, h] natural (DVE eviction adds broadcast bias).
  - scores computed TRANSPOSED: sT [k, q] = kT_chunk^T @ qT, so the exp'd
    scores are directly usable as lhsT for the second matmul with no
    transposes.  Softmax denominator comes for free from a ones-column
    appended to v (column H sums exp weights).  No max subtraction (scores
    are O(1) for this problem's scale).
  - q/k projections and scores in float32r (full-rate PE, ~tf32 multiply,
    fp32 accumulate); the attention-weights matmul in bf16 (exp output and
    v are fp16) — fp16's 10 mantissa bits keep this near fp32r error.
  - phase A software-pipelined: next q-block's transposes+projection are
    interleaved into the current block's attn matmuls so transpose-mode
    (HAM-invisible) never runs long enough to re-throttle the PE clock.
"""

import sys

sys.path.insert(0, "/opt/trn_rl_repo")

import numpy as np

import concourse.bacc as bacc
import concourse.mybir as mybir
import concourse.tile as tile

F32 = mybir.dt.float32
F32R = mybir.dt.float32r
FP16 = mybir.dt.float16


def r(ap):
    return ap.bitcast(F32R)


def build_attention_nc(B_PER_CORE, S, D, QB=512):
    """Build the per-core Bass kernel. S = seq len, D = model dim = head dim."""
    assert D % 128 == 0 and S % 512 == 0 and QB % 128 == 0 and S % QB == 0
    HC = D // 128          # chunks of the model/head dim
    KC = S // 128          # 128-row chunks of the key sequence
    KBLK = S // 512        # 512-row key blocks (phase P granularity)
    NQB = S // QB          # q blocks
    QT_PER_B = QB // 128   # 128-row q tiles per q block
    SCALE = float(1.0 / np.sqrt(np.float32(D)))

    nc = bacc.Bacc("TRN2", target_bir_lowering=False, debug=False)

    x_to = nc.declare_dram_parameter("x_to", [B_PER_CORE, S, D], F32, isOutput=False).ap()
    x_from = nc.declare_dram_parameter("x_from", [B_PER_CORE, S, D], F32, isOutput=False).ap()
    wq = nc.declare_dram_parameter("Wq", [D, D], F32, isOutput=False).ap()
    wk = nc.declare_dram_parameter("Wk", [D, D], F32, isOutput=False).ap()
    wv = nc.declare_dram_parameter("Wv", [D, D], F32, isOutput=False).ap()
    bq_pk = nc.declare_dram_parameter("bq_pk", [128, HC], F32, isOutput=False).ap()
    bk_pk = nc.declare_dram_parameter("bk_pk", [128, HC], F32, isOutput=False).ap()
    bv_b = nc.declare_dram_parameter("bv_b", [128, D + 2], F32, isOutput=False).ap()
    ident = nc.declare_dram_parameter("ident", [128, 128], F32, isOutput=False).ap()
    out = nc.declare_dram_parameter("out", [B_PER_CORE, S, D], F32, isOutput=True).ap()

    with tile.TileContext(nc) as tc:
        import contextlib

        with contextlib.ExitStack() as ctx:
            const = ctx.enter_context(tc.tile_pool(name="const", bufs=1))
            work = ctx.enter_context(tc.tile_pool(name="work", bufs=1))
            psum = ctx.enter_context(tc.tile_pool(name="psum", bufs=1, space="PSUM"))

            # ---- constants ----
            id_sb = const.tile([128, 128], F32R, name="id_sb")
            nc.sync.dma_start(out=id_sb[:], in_=r(ident[:]))
            bq_sb = const.tile([128, HC], F32, name="bq_sb")
            nc.sync.dma_start(out=bq_sb[:], in_=bq_pk[:])
            bk_sb = const.tile([128, HC], F32, name="bk_sb")
            nc.sync.dma_start(out=bk_sb[:], in_=bk_pk[:])
            bvb_sb = const.tile([128, D + 2], F32, name="bvb_sb")
            nc.sync.dma_start(out=bvb_sb[:], in_=bv_b[:])

            wq_sb, wk_sb, wv_sb = [], [], []
            for d in range(HC):
                wqt = const.tile([128, D], F32R, name=f"wq{d}")
                nc.sync.dma_start(out=wqt[:], in_=r(wq[d * 128:(d + 1) * 128, :]))
                wq_sb.append(wqt)
                wkt = const.tile([128, D], F32R, name=f"wk{d}")
                nc.sync.dma_start(out=wkt[:], in_=r(wk[d * 128:(d + 1) * 128, :]))
                wk_sb.append(wkt)
                wvt = const.tile([128, D], F32R, name=f"wv{d}")
                nc.sync.dma_start(out=wvt[:], in_=r(wv[d * 128:(d + 1) * 128, :]))
                wv_sb.append(wvt)

            # free-dim splits for matmul outputs (PSUM bank = 512 f32).
            # v projection writes [0:D); attn output writes [0:D+1) — its
            # last region includes the ones-column denominator at col D.
            d_splits = [(i, min(512, D - i)) for i in range(0, D, 512)]
            o_splits = [(i, min(512, D + 2 - i)) for i in range(0, D + 2, 512)]

            for b in range(B_PER_CORE):
                # ======== Phase P: k-side (x_from -> kT, v_ext) ========
                kT = [work.tile([128, S], F32R, name="kT", bufs=HC) for _ in range(HC)]
                vts = []
                for kb in range(KBLK):
                    # per 128-row chunk: load, transpose, then immediately
                    # project v for that chunk — interleaving matmuls between
                    # transpose bursts keeps the HAM clock gate warm
                    # (transpose-mode doesn't count as PE-busy for HAM).
                    xfT = [work.tile([128, 512], F32R, name="xT", bufs=HC + 1)
                           for _ in range(HC)]
                    for j in range(4):
                        row0 = kb * 512 + j * 128
                        xn = work.tile([128, D], F32R, name="xn", bufs=3)
                        nc.sync.dma_start(out=xn[:], in_=r(x_from[b, row0:row0 + 128, :]))
                        for d in range(HC):
                            pt = psum.tile([128, 128], F32R, name="ps_a", bufs=4)
                            nc.tensor.transpose(pt[:], xn[:, d * 128:(d + 1) * 128], id_sb[:])
                            dst = xfT[d][:, j * 128:(j + 1) * 128]
                            if d % 2 == 0:
                                nc.scalar.copy(out=dst, in_=pt[:])
                            else:
                                nc.vector.tensor_copy(out=dst, in_=pt[:])
                        pv = psum.tile([128, D + 2], F32, name="ps_o", bufs=2)
                        for (c0, cw) in d_splits:
                            for d in range(HC):
                                nc.tensor.matmul(
                                    pv[:, c0:c0 + cw],
                                    xfT[d][:, j * 128:(j + 1) * 128],
                                    wv_sb[d][:, c0:c0 + cw],
                                    start=(d == 0), stop=(d == HC - 1),
                                )
                        vt = work.tile([128, D + 2], FP16, name="v", bufs=KC)
                        nc.vector.tensor_add(vt[:, :D], pv[:, :D], bvb_sb[:, :D])
                        nc.vector.tensor_copy(out=vt[:, D:D + 2], in_=bvb_sb[:, D:D + 2])
                        vts.append(vt)
                        if j % 2 == 1:
                            # kT projection for the finished half-block
                            c0 = kb * 512 + (j - 1) * 128
                            for h in range(HC):
                                pk = psum.tile([128, 256], F32, name="ps_a", bufs=4)
                                for d in range(HC):
                                    nc.tensor.matmul(
                                        pk[:],
                                        wk_sb[d][:, h * 128:(h + 1) * 128],
                                        xfT[d][:, (j - 1) * 128:(j + 1) * 128],
                                        start=(d == 0), stop=(d == HC - 1),
                                    )
                                nc.scalar.activation(
                                    out=kT[h][:, c0:c0 + 256], in_=pk[:],
                                    func=mybir.ActivationFunctionType.Identity,
                                    bias=bk_sb[:, h:h + 1],
                                )

                # ======== Phase A: q blocks (software-pipelined) ========
                # qT for block j+1 is prepared (transpose + projection) in
                # the middle of block j's attn matmuls, so transpose-mode
                # bursts never exceed ~1us of HAM-invisible PE time and the
                # block boundary has no dependency stall.
                def prep_q_chunk(qb, j, xqT):
                    """DMA + transpose one 128-row chunk of x_to block qb."""
                    row0 = qb * QB + j * 128
                    xn = work.tile([128, D], F32R, name="xn", bufs=3)
                    nc.sync.dma_start(out=xn[:], in_=r(x_to[b, row0:row0 + 128, :]))
                    for d in range(HC):
                        pt = psum.tile([128, 128], F32R, name="ps_a", bufs=4)
                        nc.tensor.transpose(pt[:], xn[:, d * 128:(d + 1) * 128], id_sb[:])
                        dst = xqT[d][:, j * 128:(j + 1) * 128]
                        if d % 2 == 0:
                            nc.scalar.copy(out=dst, in_=pt[:])
                        else:
                            nc.vector.tensor_copy(out=dst, in_=pt[:])

                def proj_q(xqT):
                    qT = [work.tile([128, QB], F32R, name="qT", bufs=2 * HC - 2)
                          for _ in range(HC)]
                    for h in range(HC):
                        pq = psum.tile([128, QB], F32, name="ps_a", bufs=4)
                        for d in range(HC):
                            nc.tensor.matmul(
                                pq[:],
                                wq_sb[d][:, h * 128:(h + 1) * 128],
                                xqT[d][:],
                                start=(d == 0), stop=(d == HC - 1),
                            )
                        nc.scalar.activation(
                            out=qT[h][:], in_=pq[:],
                            func=mybir.ActivationFunctionType.Identity,
                            bias=bq_sb[:, h:h + 1],
                        )
                    return qT

                def new_xqT():
                    return [work.tile([128, QB], F32R, name="xT", bufs=HC + 1)
                            for _ in range(HC)]

                # prologue: prepare block 0
                xqT = new_xqT()
                for j in range(QB // 128):
                    prep_q_chunk(0, j, xqT)
                qT = proj_q(xqT)

                for qb in range(NQB):
                    q0 = qb * QB
                    # transposed scores + fused scale/exp eviction
                    ex = [work.tile([128, QB], FP16, name="expT", bufs=KC)
                          for _ in range(KC)]
                    for kc in range(KC):
                        ps = psum.tile([128, QB], F32, name="ps_a", bufs=4)
                        for h in range(HC):
                            nc.tensor.matmul(
                                ps[:],
                                kT[h][:, kc * 128:(kc + 1) * 128],
                                qT[h][:],
                                start=(h == 0), stop=(h == HC - 1),
                            )
                        nc.scalar.activation(
                            out=ex[kc][:], in_=ps[:],
                            func=mybir.ActivationFunctionType.Exp,
                            scale=SCALE,
                        )
                    # attn @ v_ext (+ denominator column), interleaved with
                    # next block's qT preparation; normalize, store
                    if qb + 1 < NQB:
                        xqT = new_xqT()
                    for t in range(QT_PER_B):
                        po = psum.tile([128, D + 2], F32, name="ps_o", bufs=2)
                        # kc-major so consecutive matmuls share the stationary
                        # operand (the two column regions live in different
                        # PSUM banks, so the accumulation groups may interleave)
                        for kc in range(KC):
                            for (c0, cw) in o_splits:
                                nc.tensor.matmul(
                                    po[:, c0:c0 + cw],
                                    ex[kc][:, t * 128:(t + 1) * 128],
                                    vts[kc][:, c0:c0 + cw],
                                    start=(kc == 0), stop=(kc == KC - 1),
                                )
                        if qb + 1 < NQB and t < QB // 128:
                            prep_q_chunk(qb + 1, t, xqT)
                        rec = work.tile([128, 1], F32, name="rec", bufs=4)
                        nc.vector.reciprocal(rec[:], po[:, D:D + 1])
                        ot = work.tile([128, D], F32, name="ot", bufs=2)
                        nc.vector.tensor_scalar_mul(ot[:], po[:, :D], rec[:])
                        row0 = q0 + t * 128
                        nc.sync.dma_start(out=out[b, row0:row0 + 128, :], in_=ot[:])
                    if qb + 1 < NQB:
                        qT = proj_q(xqT)

    nc.compile()
    return nc


def _host_inputs(x_to, x_from, Wq, bq, Wk, bk, Wv, bv, n_cores, b_per_core, D):
    HC = D // 128
    f32 = np.float32
    bq = np.asarray(bq, f32).reshape(HC, 128).T.copy()
    bk = np.asarray(bk, f32).reshape(HC, 128).T.copy()
    bv_ext = np.concatenate([np.asarray(bv, f32), np.array([1.0, 0.0], f32)])
    bv_b = np.tile(bv_ext[None, :], (128, 1)).copy()
    ident = np.eye(128, dtype=f32)
    Wq = np.ascontiguousarray(Wq, f32)
    Wk = np.ascontiguousarray(Wk, f32)
    Wv = np.ascontiguousarray(Wv, f32)
    x_to = np.asarray(x_to, f32)
    x_from = np.asarray(x_from, f32)
    in_maps = []
    for c in range(n_cores):
        lo, hi = c * b_per_core, (c + 1) * b_per_core
        in_maps.append({
            "x_to": np.ascontiguousarray(x_to[lo:hi]),
            "x_from": np.ascontiguousarray(x_from[lo:hi]),
            "Wq": Wq, "Wk": Wk, "Wv": Wv,
            "bq_pk": bq, "bk_pk": bk, "bv_b": bv_b, "ident": ident,
        })
    return in_maps


_NC_CACHE = {}


def run(x_to, x_from, Wq, bq, Wk, bk, Wv, bv, trace=False, trace_kwargs=None,
        tmpdir=None):
    from concourse.bass_utils import run_bass_kernel_spmd

    B, S, D = np.asarray(x_to).shape
    N_CORES = 8
    assert B % N_CORES == 0
    BPC = B // N_CORES

    key = (BPC, S, D)
    if key not in _NC_CACHE:
        _NC_CACHE[key] = build_attention_nc(BPC, S, D)
    nc = _NC_CACHE[key]

    in_maps = _host_inputs(x_to, x_from, Wq, bq, Wk, bk, Wv, bv, N_CORES, BPC, D)
    res = run_bass_kernel_spmd(
        nc, in_maps, list(range(N_CORES)), trace=trace,
        trace_kwargs=trace_kwargs or {}, tmpdir=tmpdir,
    )
    outp = np.concatenate([res.results[i]["out"] for i in range(N_CORES)], axis=0)
    return outp, res


def kernel(x_to, x_from, Wq, bq, Wk, bk, Wv, bv):
    outp, _ = run(x_to, x_from, Wq, bq, Wk, bk, Wv, bv)
    return outp


# revision 22
# speedup vs baseline: 1.2175x; 1.2175x over previous
"""Bass/Tile attention kernel for trn2, data-parallel over batch on 8 cores.

Computes, per batch b:
    q = x_to @ Wq + bq ; k = x_from @ Wk + bk ; v = x_from @ Wv + bv
    out = softmax(q k^T / sqrt(H)) @ v

Per-core layout strategy (2 batches per core):
  - All matmul operands fp16 (x and W rounded on host; fp32 PSUM
    accumulation).  Modeled end-to-end error vs the fp32 reference is
    ~3.5e-4 of the output absmax — the softmax averaging washes out
    elementwise rounding.
  - x transposed on PE (identity matmul, fp16 fast-weight-load) into
    xT [d, seq] tiles.
  - Scores fused: scores = x_to (Wq Wk^T) x_from^T with G = Wq Wk^T
    precomputed on host (0.14% of total FLOPs), so only ONE projection
    (uT = G x_from^T) is needed instead of two.  Valid when bq = bk = 0
    (true for this problem); otherwise falls back to separate q/k
    projections.
  - Scores computed TRANSPOSED: sT[k, q] = uT_chunk^T @ x_toT, so the
    exp'd scores feed the second matmul as lhsT with no transposes.
    Softmax denominator comes free from a ones-column appended to v
    (column D of the attn output accumulates the exp sum).  No max
    subtraction (scores are O(1) at this problem's scale).
  - Software-pipelined: the next q-block's transposes run in the middle
    of the current block's attn matmuls, and a dummy-matmul warmup keeps
    the PE HAM clock gate at 8/8 from the first real matmul on.
"""

import sys

sys.path.insert(0, "/opt/trn_rl_repo")

import numpy as np

import concourse.bacc as bacc
import concourse.mybir as mybir
import concourse.tile as tile

F32 = mybir.dt.float32
FP16 = mybir.dt.float16


def build_attention_nc(B_PER_CORE, S, D, QB=512, fuse_scores=True):
    """Build the per-core Bass kernel. S = seq len, D = model dim = head dim."""
    assert D % 128 == 0 and S % 512 == 0 and QB % 128 == 0 and S % QB == 0
    HC = D // 128          # chunks of the model/head dim
    KC = S // 128          # 128-row chunks of the key sequence
    KBLK = S // 512        # 512-row key blocks (phase P granularity)
    NQB = S // QB          # q blocks
    QT_PER_B = QB // 128   # 128-row q tiles per q block
    NCHUNK = QB // 128     # x_to chunks per q block
    SCALE = float(1.0 / np.sqrt(np.float32(D)))

    nc = bacc.Bacc("TRN2", target_bir_lowering=False, debug=False)

    x_to = nc.declare_dram_parameter("x_to", [B_PER_CORE, S, D], FP16, isOutput=False).ap()
    x_from = nc.declare_dram_parameter("x_from", [B_PER_CORE, S, D], FP16, isOutput=False).ap()
    if fuse_scores:
        # Gt = (Wq @ Wk^T)^T, host-precomputed
        gt = nc.declare_dram_parameter("Gt", [D, D], FP16, isOutput=False).ap()
    else:
        wq = nc.declare_dram_parameter("Wq", [D, D], FP16, isOutput=False).ap()
        wk = nc.declare_dram_parameter("Wk", [D, D], FP16, isOutput=False).ap()
        bq_pk = nc.declare_dram_parameter("bq_pk", [128, HC], F32, isOutput=False).ap()
        bk_pk = nc.declare_dram_parameter("bk_pk", [128, HC], F32, isOutput=False).ap()
    wv = nc.declare_dram_parameter("Wv", [D, D], FP16, isOutput=False).ap()
    bv_b = nc.declare_dram_parameter("bv_b", [128, D + 2], F32, isOutput=False).ap()
    ident = nc.declare_dram_parameter("ident", [128, 128], FP16, isOutput=False).ap()
    out = nc.declare_dram_parameter("out", [B_PER_CORE, S, D], F32, isOutput=True).ap()

    with tile.TileContext(nc) as tc:
        import contextlib

        with contextlib.ExitStack() as ctx:
            const = ctx.enter_context(tc.tile_pool(name="const", bufs=1))
            work = ctx.enter_context(tc.tile_pool(name="work", bufs=1))
            psum = ctx.enter_context(tc.tile_pool(name="psum", bufs=1, space="PSUM"))

            # ---- constants (small, front of the DMA queues) ----
            id_sb = const.tile([128, 128], FP16, name="id_sb")
            nc.sync.dma_start(out=id_sb[:], in_=ident[:])
            if not fuse_scores:
                bq_sb = const.tile([128, HC], F32, name="bq_sb")
                nc.sync.dma_start(out=bq_sb[:], in_=bq_pk[:])
                bk_sb = const.tile([128, HC], F32, name="bk_sb")
                nc.sync.dma_start(out=bk_sb[:], in_=bk_pk[:])
            bvb_sb = const.tile([128, D + 2], F32, name="bvb_sb")
            nc.gpsimd.dma_start(out=bvb_sb[:], in_=bv_b[:])

            # PE warm-up: ~3.5us of dummy matmuls on a zeroed tile so the
            # HAM clock gate reaches 8/8 before the first real matmul.
            warm = const.tile([128, 128], FP16, name="warm")
            nc.gpsimd.memset(warm[:], 0.0)
            pw = psum.tile([128, 128], F32, name="ps_a", bufs=4)
            for i in range(64):
                nc.tensor.matmul(pw[:], warm[:], warm[:],
                                 start=(i == 0), stop=(i == 63))

            # weight tiles, loaded lazily (emitted after the first x chunk so
            # the x DMA gets the front of the completion-semaphore lanes)
            wg_sb, wv_sb, wq_sb = [], [], []

            def load_weights():
                for d in range(HC):
                    wvt = const.tile([128, D], FP16, name=f"wv{d}")
                    nc.gpsimd.dma_start(out=wvt[:], in_=wv[d * 128:(d + 1) * 128, :])
                    wv_sb.append(wvt)
                    if fuse_scores:
                        wgt = const.tile([128, D], FP16, name=f"wg{d}")
                        nc.gpsimd.dma_start(out=wgt[:], in_=gt[d * 128:(d + 1) * 128, :])
                        wg_sb.append(wgt)
                    else:
                        wkt = const.tile([128, D], FP16, name=f"wk{d}")
                        nc.gpsimd.dma_start(out=wkt[:], in_=wk[d * 128:(d + 1) * 128, :])
                        wg_sb.append(wkt)
                        wqt = const.tile([128, D], FP16, name=f"wq{d}")
                        nc.gpsimd.dma_start(out=wqt[:], in_=wq[d * 128:(d + 1) * 128, :])
                        wq_sb.append(wqt)

            # free-dim splits for matmul outputs (PSUM bank = 512 f32).
            d_splits = [(i, min(512, D - i)) for i in range(0, D, 512)]
            o_splits = [(i, min(512, D + 2 - i)) for i in range(0, D + 2, 512)]

            for b in range(B_PER_CORE):
                # uT = G @ x_from^T (fused) or kT = Wk^T x_from^T (fallback):
                # either way the scores lhsT, [D, S] in HC tiles.
                uT = [work.tile([128, S], FP16, name="uT", bufs=HC + 1)
                      for _ in range(HC)]
                vts = []

                # -- helpers for preparing a q block's x_to transposes;
                #    used by the phase A pipeline and the P->A boundary --
                def prep_q_chunk(qb, j, xqT):
                    row0 = qb * QB + j * 128
                    xn = work.tile([128, D], FP16, name="xn", bufs=6)
                    nc.sync.dma_start(out=xn[:], in_=x_to[b, row0:row0 + 128, :])
                    for d in range(HC):
                        pt = psum.tile([128, 128], FP16, name="ps_a", bufs=4)
                        nc.tensor.transpose(pt[:], xn[:, d * 128:(d + 1) * 128], id_sb[:])
                        dst = xqT[d][:, j * 128:(j + 1) * 128]
                        if d % 2 == 0:
                            nc.scalar.copy(out=dst, in_=pt[:])
                        else:
                            nc.vector.tensor_copy(out=dst, in_=pt[:])

                def new_xqT():
                    return [work.tile([128, QB], FP16, name="xqT", bufs=HC + 2)
                            for _ in range(HC)]

                def proj_q(xqT):
                    """Unfused fallback: qT = Wq^T x_to^T + bq."""
                    qT = [work.tile([128, QB], FP16, name="qT", bufs=2 * HC)
                          for _ in range(HC)]
                    for h in range(HC):
                        pq = psum.tile([128, QB], F32, name="ps_a", bufs=4)
                        for d in range(HC):
                            nc.tensor.matmul(
                                pq[:],
                                wq_sb[d][:, h * 128:(h + 1) * 128],
                                xqT[d][:],
                                start=(d == 0), stop=(d == HC - 1),
                            )
                        nc.scalar.activation(
                            out=qT[h][:], in_=pq[:],
                            func=mybir.ActivationFunctionType.Identity,
                            bias=bq_sb[:, h:h + 1],
                        )
                    return qT

                # ======== Phase P: x_from -> uT (or kT), v_ext ========
                xqT = None
                for kb in range(KBLK):
                    if kb == KBLK - 1:
                        xqT = new_xqT()
                    xfT = [work.tile([128, 512], FP16, name="xT", bufs=HC + 2)
                           for _ in range(HC)]
                    for j in range(4):
                        row0 = kb * 512 + j * 128
                        xn = work.tile([128, D], FP16, name="xn", bufs=6)
                        nc.sync.dma_start(out=xn[:], in_=x_from[b, row0:row0 + 128, :])
                        for d in range(HC):
                            pt = psum.tile([128, 128], FP16, name="ps_a", bufs=4)
                            nc.tensor.transpose(pt[:], xn[:, d * 128:(d + 1) * 128], id_sb[:])
                            dst = xfT[d][:, j * 128:(j + 1) * 128]
                            if d % 2 == 0:
                                nc.scalar.copy(out=dst, in_=pt[:])
                            else:
                                nc.vector.tensor_copy(out=dst, in_=pt[:])
                        if not wv_sb:
                            load_weights()
                        # v projection for this 128-row chunk
                        pv = psum.tile([128, D + 2], F32, name="ps_o", bufs=2)
                        for (c0, cw) in d_splits:
                            for d in range(HC):
                                nc.tensor.matmul(
                                    pv[:, c0:c0 + cw],
                                    xfT[d][:, j * 128:(j + 1) * 128],
                                    wv_sb[d][:, c0:c0 + cw],
                                    start=(d == 0), stop=(d == HC - 1),
                                )
                        vt = work.tile([128, D + 2], FP16, name="v", bufs=KC + 2)
                        nc.vector.tensor_add(vt[:, :D], pv[:, :D], bvb_sb[:, :D])
                        nc.vector.tensor_copy(out=vt[:, D:D + 2], in_=bvb_sb[:, D:D + 2])
                        vts.append(vt)
                        # interleave block-0 q prep into the last phase-P block
                        if kb == KBLK - 1 and j >= 4 - NCHUNK:
                            prep_q_chunk(0, j - (4 - NCHUNK), xqT)
                        if j % 2 == 1:
                            # uT/kT projection for the finished half-block
                            c0 = kb * 512 + (j - 1) * 128
                            for h in range(HC):
                                pk = psum.tile([128, 256], F32, name="ps_a", bufs=4)
                                for d in range(HC):
                                    nc.tensor.matmul(
                                        pk[:],
                                        wg_sb[d][:, h * 128:(h + 1) * 128],
                                        xfT[d][:, (j - 1) * 128:(j + 1) * 128],
                                        start=(d == 0), stop=(d == HC - 1),
                                    )
                                if fuse_scores:
                                    nc.scalar.copy(out=uT[h][:, c0:c0 + 256], in_=pk[:])
                                else:
                                    nc.scalar.activation(
                                        out=uT[h][:, c0:c0 + 256], in_=pk[:],
                                        func=mybir.ActivationFunctionType.Identity,
                                        bias=bk_sb[:, h:h + 1],
                                    )

                # ======== Phase A: q blocks (software-pipelined) ========
                sc_rhs = xqT if fuse_scores else proj_q(xqT)

                for qb in range(NQB):
                    q0 = qb * QB
                    # transposed scores + fused scale/exp eviction
                    ex = [work.tile([128, QB], FP16, name="expT", bufs=KC + 2)
                          for _ in range(KC)]
                    for kc in range(KC):
                        ps = psum.tile([128, QB], F32, name="ps_a", bufs=4)
                        for h in range(HC):
                            nc.tensor.matmul(
                                ps[:],
                                uT[h][:, kc * 128:(kc + 1) * 128],
                                sc_rhs[h][:],
                                start=(h == 0), stop=(h == HC - 1),
                            )
                        nc.scalar.activation(
                            out=ex[kc][:], in_=ps[:],
                            func=mybir.ActivationFunctionType.Exp,
                            scale=SCALE,
                        )
                    # attn @ v_ext (+ denominator column), interleaved with
                    # the next block's x_to transposes; normalize, store
                    if qb + 1 < NQB:
                        xqT = new_xqT()
                    for t in range(QT_PER_B):
                        po = psum.tile([128, D + 2], F32, name="ps_o", bufs=2)
                        for kc in range(KC):
                            for (c0, cw) in o_splits:
                                nc.tensor.matmul(
                                    po[:, c0:c0 + cw],
                                    ex[kc][:, t * 128:(t + 1) * 128],
                                    vts[kc][:, c0:c0 + cw],
                                    start=(kc == 0), stop=(kc == KC - 1),
                                )
                        if qb + 1 < NQB and t < NCHUNK:
                            prep_q_chunk(qb + 1, t, xqT)
                        rec = work.tile([128, 1], F32, name="rec", bufs=4)
                        nc.vector.reciprocal(rec[:], po[:, D:D + 1])
                        ot = work.tile([128, D], F32, name="ot", bufs=3)
                        nc.vector.tensor_scalar_mul(ot[:], po[:, :D], rec[:])
                        row0 = q0 + t * 128
                        nc.sync.dma_start(out=out[b, row0:row0 + 128, :], in_=ot[:])
                    if qb + 1 < NQB:
                        sc_rhs = xqT if fuse_scores else proj_q(xqT)

    nc.compile()
    return nc


def _host_inputs(x_to, x_from, Wq, bq, Wk, bk, Wv, bv, n_cores, b_per_core, D,
                 fuse_scores):
    HC = D // 128
    f32, f16 = np.float32, np.float16
    bv_ext = np.concatenate([np.asarray(bv, f32), np.array([1.0, 0.0], f32)])
    bv_b = np.tile(bv_ext[None, :], (128, 1)).copy()
    ident = np.eye(128, dtype=f16)
    Wv16 = np.ascontiguousarray(Wv, f16)
    x_to = np.asarray(x_to, f16)
    x_from = np.asarray(x_from, f16)
    common = {"Wv": Wv16, "bv_b": bv_b, "ident": ident}
    if fuse_scores:
        G = np.asarray(Wq, np.float64) @ np.asarray(Wk, np.float64).T
        common["Gt"] = np.ascontiguousarray(G.T, f16)
    else:
        common["Wq"] = np.ascontiguousarray(Wq, f16)
        common["Wk"] = np.ascontiguousarray(Wk, f16)
        common["bq_pk"] = np.asarray(bq, f32).reshape(HC, 128).T.copy()
        common["bk_pk"] = np.asarray(bk, f32).reshape(HC, 128).T.copy()
    in_maps = []
    for c in range(n_cores):
        lo, hi = c * b_per_core, (c + 1) * b_per_core
        in_maps.append({
            "x_to": np.ascontiguousarray(x_to[lo:hi]),
            "x_from": np.ascontiguousarray(x_from[lo:hi]),
            **common,
        })
    return in_maps


_NC_CACHE = {}


def run(x_to, x_from, Wq, bq, Wk, bk, Wv, bv, trace=False, trace_kwargs=None,
        tmpdir=None):
    from concourse.bass_utils import run_bass_kernel_spmd

    B, S, D = np.asarray(x_to).shape
    N_CORES = 8
    assert B % N_CORES == 0
    BPC = B // N_CORES

    fuse = bool(np.all(np.asarray(bq) == 0) and np.all(np.asarray(bk) == 0))
    key = (BPC, S, D, fuse)
    if key not in _NC_CACHE:
        _NC_CACHE[key] = build_attention_nc(BPC, S, D, fuse_scores=fuse)
    nc = _NC_CACHE[key]

    in_maps = _host_inputs(x_to, x_from, Wq, bq, Wk, bk, Wv, bv, N_CORES, BPC, D,
                           fuse)
    res = run_bass_kernel_spmd(
        nc, in_maps, list(range(N_CORES)), trace=trace,
        trace_kwargs=trace_kwargs or {}, tmpdir=tmpdir,
    )
    outp = np.concatenate([res.results[i]["out"] for i in range(N_CORES)], axis=0)
    return outp, res


def kernel(x_to, x_from, Wq, bq, Wk, bk, Wv, bv):
    outp, _ = run(x_to, x_from, Wq, bq, Wk, bk, Wv, bv)
    return outp


# revision 24
# speedup vs baseline: 1.2224x; 1.0040x over previous
"""Bass/Tile attention kernel for trn2, data-parallel over batch on 8 cores.

Computes, per batch b:
    q = x_to @ Wq + bq ; k = x_from @ Wk + bk ; v = x_from @ Wv + bv
    out = softmax(q k^T / sqrt(H)) @ v

Per-core layout strategy (2 batches per core):
  - All matmul operands fp16 (x and W rounded on host; fp32 PSUM
    accumulation).  Modeled end-to-end error vs the fp32 reference is
    ~3.5e-4 of the output absmax — the softmax averaging washes out
    elementwise rounding.
  - x transposed on PE (identity matmul, fp16 fast-weight-load) into
    xT [d, seq] tiles.
  - Scores fused: scores = x_to (Wq Wk^T) x_from^T with G = Wq Wk^T
    precomputed on host (0.14% of total FLOPs), so only ONE projection
    (uT = G x_from^T) is needed instead of two.  Valid when bq = bk = 0
    (true for this problem); otherwise falls back to separate q/k
    projections.
  - Scores computed TRANSPOSED: sT[k, q] = uT_chunk^T @ x_toT, so the
    exp'd scores feed the second matmul as lhsT with no transposes.
    Softmax denominator comes free from a ones-column appended to v
    (column D of the attn output accumulates the exp sum).  No max
    subtraction (scores are O(1) at this problem's scale).
  - Software-pipelined: the next q-block's transposes run in the middle
    of the current block's attn matmuls, and a dummy-matmul warmup keeps
    the PE HAM clock gate at 8/8 from the first real matmul on.
"""

import sys

sys.path.insert(0, "/opt/trn_rl_repo")

import numpy as np

import concourse.bacc as bacc
import concourse.mybir as mybir
import concourse.tile as tile

F32 = mybir.dt.float32
FP16 = mybir.dt.float16


def build_attention_nc(B_PER_CORE, S, D, QB=512, fuse_scores=True):
    """Build the per-core Bass kernel. S = seq len, D = model dim = head dim."""
    assert D % 128 == 0 and S % 512 == 0 and QB % 128 == 0 and S % QB == 0
    HC = D // 128          # chunks of the model/head dim
    KC = S // 128          # 128-row chunks of the key sequence
    KBLK = S // 512        # 512-row key blocks (phase P granularity)
    NQB = S // QB          # q blocks
    QT_PER_B = QB // 128   # 128-row q tiles per q block
    NCHUNK = QB // 128     # x_to chunks per q block
    SCALE = float(1.0 / np.sqrt(np.float32(D)))

    nc = bacc.Bacc("TRN2", target_bir_lowering=False, debug=False)

    x_to = nc.declare_dram_parameter("x_to", [B_PER_CORE, S, D], FP16, isOutput=False).ap()
    x_from = nc.declare_dram_parameter("x_from", [B_PER_CORE, S, D], FP16, isOutput=False).ap()
    if fuse_scores:
        # Gt = (Wq @ Wk^T)^T, host-precomputed
        gt = nc.declare_dram_parameter("Gt", [D, D], FP16, isOutput=False).ap()
    else:
        wq = nc.declare_dram_parameter("Wq", [D, D], FP16, isOutput=False).ap()
        wk = nc.declare_dram_parameter("Wk", [D, D], FP16, isOutput=False).ap()
        bq_pk = nc.declare_dram_parameter("bq_pk", [128, HC], F32, isOutput=False).ap()
        bk_pk = nc.declare_dram_parameter("bk_pk", [128, HC], F32, isOutput=False).ap()
    wv = nc.declare_dram_parameter("Wv", [D, D], FP16, isOutput=False).ap()
    bv_b = nc.declare_dram_parameter("bv_b", [128, D + 2], F32, isOutput=False).ap()
    ident = nc.declare_dram_parameter("ident", [128, 128], FP16, isOutput=False).ap()
    out = nc.declare_dram_parameter("out", [B_PER_CORE, S, D], F32, isOutput=True).ap()

    with tile.TileContext(nc) as tc:
        import contextlib

        with contextlib.ExitStack() as ctx:
            const = ctx.enter_context(tc.tile_pool(name="const", bufs=1))
            work = ctx.enter_context(tc.tile_pool(name="work", bufs=1))
            psum = ctx.enter_context(tc.tile_pool(name="psum", bufs=1, space="PSUM"))

            # ---- constants (small, front of the DMA queues) ----
            id_sb = const.tile([128, 128], FP16, name="id_sb")
            nc.sync.dma_start(out=id_sb[:], in_=ident[:])
            if not fuse_scores:
                bq_sb = const.tile([128, HC], F32, name="bq_sb")
                nc.sync.dma_start(out=bq_sb[:], in_=bq_pk[:])
                bk_sb = const.tile([128, HC], F32, name="bk_sb")
                nc.sync.dma_start(out=bk_sb[:], in_=bk_pk[:])
            bvb_sb = const.tile([128, D + 2], F32, name="bvb_sb")
            nc.gpsimd.dma_start(out=bvb_sb[:], in_=bv_b[:])

            # PE warm-up: ~3.5us of dummy matmuls on a zeroed tile so the
            # HAM clock gate reaches 8/8 before the first real matmul.
            warm = const.tile([128, 128], FP16, name="warm")
            nc.gpsimd.memset(warm[:], 0.0)
            pw = psum.tile([128, 128], F32, name="ps_a", bufs=4)
            for i in range(64):
                nc.tensor.matmul(pw[:], warm[:], warm[:],
                                 start=(i == 0), stop=(i == 63))

            # weight tiles, loaded lazily (emitted after the first x chunk so
            # the x DMA gets the front of the completion-semaphore lanes)
            wg_sb, wv_sb, wq_sb = [], [], []

            def load_weights():
                # one big DMA per weight matrix: [D, D] -> [128, HC*D] with
                # chunk c at columns [c*D, (c+1)*D)
                wv_all = const.tile([128, HC, D], FP16, name="wv_all")
                nc.gpsimd.dma_start(
                    out=wv_all[:], in_=wv.rearrange("(c p) h -> p c h", p=128))
                wv_sb.extend(wv_all[:, d, :] for d in range(HC))
                if fuse_scores:
                    wg_all = const.tile([128, HC, D], FP16, name="wg_all")
                    nc.gpsimd.dma_start(
                        out=wg_all[:], in_=gt.rearrange("(c p) h -> p c h", p=128))
                    wg_sb.extend(wg_all[:, d, :] for d in range(HC))
                else:
                    wk_all = const.tile([128, HC, D], FP16, name="wk_all")
                    nc.gpsimd.dma_start(
                        out=wk_all[:], in_=wk.rearrange("(c p) h -> p c h", p=128))
                    wg_sb.extend(wk_all[:, d, :] for d in range(HC))
                    wq_all = const.tile([128, HC, D], FP16, name="wq_all")
                    nc.gpsimd.dma_start(
                        out=wq_all[:], in_=wq.rearrange("(c p) h -> p c h", p=128))
                    wq_sb.extend(wq_all[:, d, :] for d in range(HC))

            # free-dim splits for matmul outputs (PSUM bank = 512 f32).
            d_splits = [(i, min(512, D - i)) for i in range(0, D, 512)]
            o_splits = [(i, min(512, D + 2 - i)) for i in range(0, D + 2, 512)]

            for b in range(B_PER_CORE):
                # uT = G @ x_from^T (fused) or kT = Wk^T x_from^T (fallback):
                # either way the scores lhsT, [D, S] in HC tiles.
                uT = [work.tile([128, S], FP16, name="uT", bufs=HC + 1)
                      for _ in range(HC)]
                vts = []

                # -- helpers for preparing a q block's x_to transposes;
                #    used by the phase A pipeline and the P->A boundary --
                def prep_q_chunk(qb, j, xqT):
                    row0 = qb * QB + j * 128
                    xn = work.tile([128, D], FP16, name="xn", bufs=8)
                    nc.sync.dma_start(out=xn[:], in_=x_to[b, row0:row0 + 128, :])
                    for d in range(HC):
                        pt = psum.tile([128, 128], FP16, name="ps_a", bufs=4)
                        nc.tensor.transpose(pt[:], xn[:, d * 128:(d + 1) * 128], id_sb[:])
                        dst = xqT[d][:, j * 128:(j + 1) * 128]
                        if d % 2 == 0:
                            nc.scalar.copy(out=dst, in_=pt[:])
                        else:
                            nc.vector.tensor_copy(out=dst, in_=pt[:])

                def new_xqT():
                    return [work.tile([128, QB], FP16, name="xqT", bufs=HC + 4)
                            for _ in range(HC)]

                def proj_q(xqT):
                    """Unfused fallback: qT = Wq^T x_to^T + bq."""
                    qT = [work.tile([128, QB], FP16, name="qT", bufs=2 * HC)
                          for _ in range(HC)]
                    for h in range(HC):
                        pq = psum.tile([128, QB], F32, name="ps_a", bufs=4)
                        for d in range(HC):
                            nc.tensor.matmul(
                                pq[:],
                                wq_sb[d][:, h * 128:(h + 1) * 128],
                                xqT[d][:],
                                start=(d == 0), stop=(d == HC - 1),
                            )
                        nc.scalar.activation(
                            out=qT[h][:], in_=pq[:],
                            func=mybir.ActivationFunctionType.Identity,
                            bias=bq_sb[:, h:h + 1],
                        )
                    return qT

                # ======== Phase P: x_from -> uT (or kT), v_ext ========
                xqT = None
                for kb in range(KBLK):
                    if kb == KBLK - 1:
                        xqT = new_xqT()
                    xfT = [work.tile([128, 512], FP16, name="xT", bufs=2 * HC)
                           for _ in range(HC)]
                    for j in range(4):
                        row0 = kb * 512 + j * 128
                        xn = work.tile([128, D], FP16, name="xn", bufs=8)
                        nc.sync.dma_start(out=xn[:], in_=x_from[b, row0:row0 + 128, :])
                        for d in range(HC):
                            pt = psum.tile([128, 128], FP16, name="ps_a", bufs=4)
                            nc.tensor.transpose(pt[:], xn[:, d * 128:(d + 1) * 128], id_sb[:])
                            dst = xfT[d][:, j * 128:(j + 1) * 128]
                            if d % 2 == 0:
                                nc.scalar.copy(out=dst, in_=pt[:])
                            else:
                                nc.vector.tensor_copy(out=dst, in_=pt[:])
                        if not wv_sb:
                            load_weights()
                        # v projection for this 128-row chunk
                        pv = psum.tile([128, D + 2], F32, name="ps_o", bufs=2)
                        for (c0, cw) in d_splits:
                            for d in range(HC):
                                nc.tensor.matmul(
                                    pv[:, c0:c0 + cw],
                                    xfT[d][:, j * 128:(j + 1) * 128],
                                    wv_sb[d][:, c0:c0 + cw],
                                    start=(d == 0), stop=(d == HC - 1),
                                )
                        vt = work.tile([128, D + 2], FP16, name="v", bufs=KC + 4)
                        nc.vector.tensor_add(vt[:, :D], pv[:, :D], bvb_sb[:, :D])
                        nc.vector.tensor_copy(out=vt[:, D:D + 2], in_=bvb_sb[:, D:D + 2])
                        vts.append(vt)
                        # interleave block-0 q prep into the last phase-P block
                        if kb == KBLK - 1 and j >= 4 - NCHUNK:
                            prep_q_chunk(0, j - (4 - NCHUNK), xqT)
                        if j % 2 == 1:
                            # uT/kT projection for the finished half-block
                            c0 = kb * 512 + (j - 1) * 128
                            for h in range(HC):
                                pk = psum.tile([128, 256], F32, name="ps_a", bufs=4)
                                for d in range(HC):
                                    nc.tensor.matmul(
                                        pk[:],
                                        wg_sb[d][:, h * 128:(h + 1) * 128],
                                        xfT[d][:, (j - 1) * 128:(j + 1) * 128],
                                        start=(d == 0), stop=(d == HC - 1),
                                    )
                                if fuse_scores:
                                    nc.scalar.copy(out=uT[h][:, c0:c0 + 256], in_=pk[:])
                                else:
                                    nc.scalar.activation(
                                        out=uT[h][:, c0:c0 + 256], in_=pk[:],
                                        func=mybir.ActivationFunctionType.Identity,
                                        bias=bk_sb[:, h:h + 1],
                                    )

                # ======== Phase A: q blocks (software-pipelined) ========
                sc_rhs = xqT if fuse_scores else proj_q(xqT)

                for qb in range(NQB):
                    q0 = qb * QB
                    # transposed scores + fused scale/exp eviction
                    ex = [work.tile([128, QB], FP16, name="expT", bufs=KC + 4)
                          for _ in range(KC)]
                    for kc in range(KC):
                        ps = psum.tile([128, QB], F32, name="ps_a", bufs=4)
                        for h in range(HC):
                            nc.tensor.matmul(
                                ps[:],
                                uT[h][:, kc * 128:(kc + 1) * 128],
                                sc_rhs[h][:],
                                start=(h == 0), stop=(h == HC - 1),
                            )
                        nc.scalar.activation(
                            out=ex[kc][:], in_=ps[:],
                            func=mybir.ActivationFunctionType.Exp,
                            scale=SCALE,
                        )
                    # attn @ v_ext (+ denominator column), interleaved with
                    # the next block's x_to transposes; normalize, store
                    if qb + 1 < NQB:
                        xqT = new_xqT()
                    for t in range(QT_PER_B):
                        po = psum.tile([128, D + 2], F32, name="ps_o", bufs=2)
                        for kc in range(KC):
                            for (c0, cw) in o_splits:
                                nc.tensor.matmul(
                                    po[:, c0:c0 + cw],
                                    ex[kc][:, t * 128:(t + 1) * 128],
                                    vts[kc][:, c0:c0 + cw],
                                    start=(kc == 0), stop=(kc == KC - 1),
                                )
                        if qb + 1 < NQB and t < NCHUNK:
                            prep_q_chunk(qb + 1, t, xqT)
                        rec = work.tile([128, 1], F32, name="rec", bufs=4)
                        nc.vector.reciprocal(rec[:], po[:, D:D + 1])
                        ot = work.tile([128, D], F32, name="ot", bufs=3)
                        nc.vector.tensor_scalar_mul(ot[:], po[:, :D], rec[:])
                        row0 = q0 + t * 128
                        nc.sync.dma_start(out=out[b, row0:row0 + 128, :], in_=ot[:])
                    if qb + 1 < NQB:
                        sc_rhs = xqT if fuse_scores else proj_q(xqT)

    nc.compile()
    return nc


def _host_inputs(x_to, x_from, Wq, bq, Wk, bk, Wv, bv, n_cores, b_per_core, D,
                 fuse_scores):
    HC = D // 128
    f32, f16 = np.float32, np.float16
    bv_ext = np.concatenate([np.asarray(bv, f32), np.array([1.0, 0.0], f32)])
    bv_b = np.tile(bv_ext[None, :], (128, 1)).copy()
    ident = np.eye(128, dtype=f16)
    Wv16 = np.ascontiguousarray(Wv, f16)
    x_to = np.asarray(x_to, f16)
    x_from = np.asarray(x_from, f16)
    common = {"Wv": Wv16, "bv_b": bv_b, "ident": ident}
    if fuse_scores:
        G = np.asarray(Wq, np.float64) @ np.asarray(Wk, np.float64).T
        common["Gt"] = np.ascontiguousarray(G.T, f16)
    else:
        common["Wq"] = np.ascontiguousarray(Wq, f16)
        common["Wk"] = np.ascontiguousarray(Wk, f16)
        common["bq_pk"] = np.asarray(bq, f32).reshape(HC, 128).T.copy()
        common["bk_pk"] = np.asarray(bk, f32).reshape(HC, 128).T.copy()
    in_maps = []
    for c in range(n_cores):
        lo, hi = c * b_per_core, (c + 1) * b_per_core
        in_maps.append({
            "x_to": np.ascontiguousarray(x_to[lo:hi]),
            "x_from": np.ascontiguousarray(x_from[lo:hi]),
            **common,
        })
    return in_maps


_NC_CACHE = {}


def run(x_to, x_from, Wq, bq, Wk, bk, Wv, bv, trace=False, trace_kwargs=None,
        tmpdir=None):
    from concourse.bass_utils import run_bass_kernel_spmd

    B, S, D = np.asarray(x_to).shape
    N_CORES = 8
    assert B % N_CORES == 0
    BPC = B // N_CORES

    fuse = bool(np.all(np.asarray(bq) == 0) and np.all(np.asarray(bk) == 0))
    key = (BPC, S, D, fuse)
    if key not in _NC_CACHE:
        _NC_CACHE[key] = build_attention_nc(BPC, S, D, fuse_scores=fuse)
    nc = _NC_CACHE[key]

    in_maps = _host_inputs(x_to, x_from, Wq, bq, Wk, bk, Wv, bv, N_CORES, BPC, D,
                           fuse)
    res = run_bass_kernel_spmd(
        nc, in_maps, list(range(N_CORES)), trace=trace,
        trace_kwargs=trace_kwargs or {}, tmpdir=tmpdir,
    )
    outp = np.concatenate([res.results[i]["out"] for i in range(N_CORES)], axis=0)
    return outp, res


def kernel(x_to, x_from, Wq, bq, Wk, bk, Wv, bv):
    outp, _ = run(x_to, x_from, Wq, bq, Wk, bk, Wv, bv)
    return outp


# revision 25
# speedup vs baseline: 1.2255x; 1.0025x over previous
"""Bass/Tile attention kernel for trn2, data-parallel over batch on 8 cores.

Computes, per batch b:
    q = x_to @ Wq + bq ; k = x_from @ Wk + bk ; v = x_from @ Wv + bv
    out = softmax(q k^T / sqrt(H)) @ v

Per-core layout strategy (2 batches per core):
  - All matmul operands fp16 (x and W rounded on host; fp32 PSUM
    accumulation).  Modeled end-to-end error vs the fp32 reference is
    ~3.5e-4 of the output absmax — the softmax averaging washes out
    elementwise rounding.
  - x transposed on PE (identity matmul, fp16 fast-weight-load) into
    xT [d, seq] tiles.
  - Scores fused: scores = x_to (Wq Wk^T) x_from^T with G = Wq Wk^T
    precomputed on host (0.14% of total FLOPs), so only ONE projection
    (uT = G x_from^T) is needed instead of two.  Valid when bq = bk = 0
    (true for this problem); otherwise falls back to separate q/k
    projections.
  - Scores computed TRANSPOSED: sT[k, q] = uT_chunk^T @ x_toT, so the
    exp'd scores feed the second matmul as lhsT with no transposes.
    Softmax denominator comes free from a ones-column appended to v
    (column D of the attn output accumulates the exp sum).  No max
    subtraction (scores are O(1) at this problem's scale).
  - Software-pipelined: the next q-block's transposes run in the middle
    of the current block's attn matmuls, and a dummy-matmul warmup keeps
    the PE HAM clock gate at 8/8 from the first real matmul on.
"""

import sys

sys.path.insert(0, "/opt/trn_rl_repo")

import numpy as np

import concourse.bacc as bacc
import concourse.mybir as mybir
import concourse.tile as tile

F32 = mybir.dt.float32
FP16 = mybir.dt.float16


def build_attention_nc(B_PER_CORE, S, D, QB=512, fuse_scores=True):
    """Build the per-core Bass kernel. S = seq len, D = model dim = head dim."""
    assert D % 128 == 0 and S % 512 == 0 and QB % 128 == 0 and S % QB == 0
    HC = D // 128          # chunks of the model/head dim
    KC = S // 128          # 128-row chunks of the key sequence
    KBLK = S // 512        # 512-row key blocks (phase P granularity)
    NQB = S // QB          # q blocks
    QT_PER_B = QB // 128   # 128-row q tiles per q block
    NCHUNK = QB // 128     # x_to chunks per q block
    SCALE = float(1.0 / np.sqrt(np.float32(D)))

    nc = bacc.Bacc("TRN2", target_bir_lowering=False, debug=False)

    x_to = nc.declare_dram_parameter("x_to", [B_PER_CORE, S, D], FP16, isOutput=False).ap()
    x_from = nc.declare_dram_parameter("x_from", [B_PER_CORE, S, D], FP16, isOutput=False).ap()
    if fuse_scores:
        # Gt = (Wq @ Wk^T)^T, host-precomputed
        gt = nc.declare_dram_parameter("Gt", [D, D], FP16, isOutput=False).ap()
    else:
        wq = nc.declare_dram_parameter("Wq", [D, D], FP16, isOutput=False).ap()
        wk = nc.declare_dram_parameter("Wk", [D, D], FP16, isOutput=False).ap()
        bq_pk = nc.declare_dram_parameter("bq_pk", [128, HC], F32, isOutput=False).ap()
        bk_pk = nc.declare_dram_parameter("bk_pk", [128, HC], F32, isOutput=False).ap()
    wv = nc.declare_dram_parameter("Wv", [D, D], FP16, isOutput=False).ap()
    bv_b = nc.declare_dram_parameter("bv_b", [128, D + 2], F32, isOutput=False).ap()
    ident = nc.declare_dram_parameter("ident", [128, 128], FP16, isOutput=False).ap()
    out = nc.declare_dram_parameter("out", [B_PER_CORE, S, D], F32, isOutput=True).ap()

    with tile.TileContext(nc) as tc:
        import contextlib

        with contextlib.ExitStack() as ctx:
            const = ctx.enter_context(tc.tile_pool(name="const", bufs=1))
            work = ctx.enter_context(tc.tile_pool(name="work", bufs=1))
            psum = ctx.enter_context(tc.tile_pool(name="psum", bufs=1, space="PSUM"))

            # ---- constants (small, front of the DMA queues) ----
            id_sb = const.tile([128, 128], FP16, name="id_sb")
            nc.sync.dma_start(out=id_sb[:], in_=ident[:])
            if not fuse_scores:
                bq_sb = const.tile([128, HC], F32, name="bq_sb")
                nc.sync.dma_start(out=bq_sb[:], in_=bq_pk[:])
                bk_sb = const.tile([128, HC], F32, name="bk_sb")
                nc.sync.dma_start(out=bk_sb[:], in_=bk_pk[:])
            # PE warm-up: dummy matmuls on a zeroed tile so the HAM clock
            # gate reaches 8/8 before the first real matmul, sized to also
            # cover the weight-DMA arrival (~17us).
            warm = const.tile([128, 128], FP16, name="warm")
            nc.gpsimd.memset(warm[:], 0.0)
            pw = psum.tile([128, 128], F32, name="ps_a", bufs=4)
            wg_sb, wv_sb, wq_sb = [], [], []

            def load_weights():
                # one big DMA per weight matrix: [D, D] -> [128, HC*D] with
                # chunk c at columns [c*D, (c+1)*D)
                wv_all = const.tile([128, HC, D], FP16, name="wv_all")
                nc.gpsimd.dma_start(
                    out=wv_all[:], in_=wv.rearrange("(c p) h -> p c h", p=128))
                wv_sb.extend(wv_all[:, d, :] for d in range(HC))
                if fuse_scores:
                    wg_all = const.tile([128, HC, D], FP16, name="wg_all")
                    nc.gpsimd.dma_start(
                        out=wg_all[:], in_=gt.rearrange("(c p) h -> p c h", p=128))
                    wg_sb.extend(wg_all[:, d, :] for d in range(HC))
                else:
                    wk_all = const.tile([128, HC, D], FP16, name="wk_all")
                    nc.gpsimd.dma_start(
                        out=wk_all[:], in_=wk.rearrange("(c p) h -> p c h", p=128))
                    wg_sb.extend(wk_all[:, d, :] for d in range(HC))
                    wq_all = const.tile([128, HC, D], FP16, name="wq_all")
                    nc.gpsimd.dma_start(
                        out=wq_all[:], in_=wq.rearrange("(c p) h -> p c h", p=128))
                    wq_sb.extend(wq_all[:, d, :] for d in range(HC))

            load_weights()
            for i in range(96):
                nc.tensor.matmul(pw[:], warm[:], warm[:],
                                 start=(i == 0), stop=(i == 95))
            bvb_sb = const.tile([128, D + 2], F32, name="bvb_sb")
            nc.gpsimd.dma_start(out=bvb_sb[:], in_=bv_b[:])

            # free-dim splits for matmul outputs (PSUM bank = 512 f32).
            d_splits = [(i, min(512, D - i)) for i in range(0, D, 512)]
            o_splits = [(i, min(512, D + 2 - i)) for i in range(0, D + 2, 512)]

            for b in range(B_PER_CORE):
                # uT = G @ x_from^T (fused) or kT = Wk^T x_from^T (fallback):
                # either way the scores lhsT, [D, S] in HC tiles.
                uT = [work.tile([128, S], FP16, name="uT", bufs=HC + 1)
                      for _ in range(HC)]
                vts = []

                # -- helpers for preparing a q block's x_to transposes;
                #    used by the phase A pipeline and the P->A boundary --
                def prep_q_chunk(qb, j, xqT):
                    row0 = qb * QB + j * 128
                    xn = work.tile([128, D], FP16, name="xn", bufs=8)
                    nc.sync.dma_start(out=xn[:], in_=x_to[b, row0:row0 + 128, :])
                    for d in range(HC):
                        pt = psum.tile([128, 128], FP16, name="ps_a", bufs=4)
                        nc.tensor.transpose(pt[:], xn[:, d * 128:(d + 1) * 128], id_sb[:])
                        dst = xqT[d][:, j * 128:(j + 1) * 128]
                        if d % 2 == 0:
                            nc.scalar.copy(out=dst, in_=pt[:])
                        else:
                            nc.vector.tensor_copy(out=dst, in_=pt[:])

                def new_xqT():
                    return [work.tile([128, QB], FP16, name="xqT", bufs=HC + 4)
                            for _ in range(HC)]

                def proj_q(xqT):
                    """Unfused fallback: qT = Wq^T x_to^T + bq."""
                    qT = [work.tile([128, QB], FP16, name="qT", bufs=2 * HC)
                          for _ in range(HC)]
                    for h in range(HC):
                        pq = psum.tile([128, QB], F32, name="ps_a", bufs=4)
                        for d in range(HC):
                            nc.tensor.matmul(
                                pq[:],
                                wq_sb[d][:, h * 128:(h + 1) * 128],
                                xqT[d][:],
                                start=(d == 0), stop=(d == HC - 1),
                            )
                        nc.scalar.activation(
                            out=qT[h][:], in_=pq[:],
                            func=mybir.ActivationFunctionType.Identity,
                            bias=bq_sb[:, h:h + 1],
                        )
                    return qT

                # ======== Phase P: x_from -> uT (or kT), v_ext ========
                xqT = None
                for kb in range(KBLK):
                    if kb == KBLK - 1:
                        xqT = new_xqT()
                    xfT = [work.tile([128, 512], FP16, name="xT", bufs=2 * HC)
                           for _ in range(HC)]
                    for j in range(4):
                        row0 = kb * 512 + j * 128
                        xn = work.tile([128, D], FP16, name="xn", bufs=8)
                        nc.sync.dma_start(out=xn[:], in_=x_from[b, row0:row0 + 128, :])
                        for d in range(HC):
                            pt = psum.tile([128, 128], FP16, name="ps_a", bufs=4)
                            nc.tensor.transpose(pt[:], xn[:, d * 128:(d + 1) * 128], id_sb[:])
                            dst = xfT[d][:, j * 128:(j + 1) * 128]
                            if d % 2 == 0:
                                nc.scalar.copy(out=dst, in_=pt[:])
                            else:
                                nc.vector.tensor_copy(out=dst, in_=pt[:])
                        # v projection for this 128-row chunk
                        pv = psum.tile([128, D + 2], F32, name="ps_o", bufs=2)
                        for (c0, cw) in d_splits:
                            for d in range(HC):
                                nc.tensor.matmul(
                                    pv[:, c0:c0 + cw],
                                    xfT[d][:, j * 128:(j + 1) * 128],
                                    wv_sb[d][:, c0:c0 + cw],
                                    start=(d == 0), stop=(d == HC - 1),
                                )
                        vt = work.tile([128, D + 2], FP16, name="v", bufs=KC + 4)
                        nc.vector.tensor_add(vt[:, :D], pv[:, :D], bvb_sb[:, :D])
                        nc.vector.tensor_copy(out=vt[:, D:D + 2], in_=bvb_sb[:, D:D + 2])
                        vts.append(vt)
                        # interleave block-0 q prep into the last phase-P block
                        if kb == KBLK - 1 and j >= 4 - NCHUNK:
                            prep_q_chunk(0, j - (4 - NCHUNK), xqT)
                        if j % 2 == 1:
                            # uT/kT projection for the finished half-block
                            c0 = kb * 512 + (j - 1) * 128
                            for h in range(HC):
                                pk = psum.tile([128, 256], F32, name="ps_a", bufs=4)
                                for d in range(HC):
                                    nc.tensor.matmul(
                                        pk[:],
                                        wg_sb[d][:, h * 128:(h + 1) * 128],
                                        xfT[d][:, (j - 1) * 128:(j + 1) * 128],
                                        start=(d == 0), stop=(d == HC - 1),
                                    )
                                if fuse_scores:
                                    if h % 2 == 0:
                                        nc.scalar.copy(out=uT[h][:, c0:c0 + 256], in_=pk[:])
                                    else:
                                        nc.vector.tensor_copy(out=uT[h][:, c0:c0 + 256], in_=pk[:])
                                else:
                                    nc.scalar.activation(
                                        out=uT[h][:, c0:c0 + 256], in_=pk[:],
                                        func=mybir.ActivationFunctionType.Identity,
                                        bias=bk_sb[:, h:h + 1],
                                    )

                # ======== Phase A: q blocks (software-pipelined) ========
                sc_rhs = xqT if fuse_scores else proj_q(xqT)

                for qb in range(NQB):
                    q0 = qb * QB
                    # transposed scores + fused scale/exp eviction
                    ex = [work.tile([128, QB], FP16, name="expT", bufs=KC + 4)
                          for _ in range(KC)]
                    for kc in range(KC):
                        ps = psum.tile([128, QB], F32, name="ps_a", bufs=4)
                        for h in range(HC):
                            nc.tensor.matmul(
                                ps[:],
                                uT[h][:, kc * 128:(kc + 1) * 128],
                                sc_rhs[h][:],
                                start=(h == 0), stop=(h == HC - 1),
                            )
                        nc.scalar.activation(
                            out=ex[kc][:], in_=ps[:],
                            func=mybir.ActivationFunctionType.Exp,
                            scale=SCALE,
                        )
                    # attn @ v_ext (+ denominator column), interleaved with
                    # the next block's x_to transposes; normalize, store
                    if qb + 1 < NQB:
                        xqT = new_xqT()
                    for t in range(QT_PER_B):
                        po = psum.tile([128, D + 2], F32, name="ps_o", bufs=2)
                        for kc in range(KC):
                            for (c0, cw) in o_splits:
                                nc.tensor.matmul(
                                    po[:, c0:c0 + cw],
                                    ex[kc][:, t * 128:(t + 1) * 128],
                                    vts[kc][:, c0:c0 + cw],
                                    start=(kc == 0), stop=(kc == KC - 1),
                                )
                        if qb + 1 < NQB and t < NCHUNK:
                            prep_q_chunk(qb + 1, t, xqT)
                        rec = work.tile([128, 1], F32, name="rec", bufs=4)
                        nc.vector.reciprocal(rec[:], po[:, D:D + 1])
                        ot = work.tile([128, D], F32, name="ot", bufs=3)
                        nc.vector.tensor_scalar_mul(ot[:], po[:, :D], rec[:])
                        row0 = q0 + t * 128
                        nc.sync.dma_start(out=out[b, row0:row0 + 128, :], in_=ot[:])
                    if qb + 1 < NQB:
                        sc_rhs = xqT if fuse_scores else proj_q(xqT)

    nc.compile()
    return nc


def _host_inputs(x_to, x_from, Wq, bq, Wk, bk, Wv, bv, n_cores, b_per_core, D,
                 fuse_scores):
    HC = D // 128
    f32, f16 = np.float32, np.float16
    bv_ext = np.concatenate([np.asarray(bv, f32), np.array([1.0, 0.0], f32)])
    bv_b = np.tile(bv_ext[None, :], (128, 1)).copy()
    ident = np.eye(128, dtype=f16)
    Wv16 = np.ascontiguousarray(Wv, f16)
    x_to = np.asarray(x_to, f16)
    x_from = np.asarray(x_from, f16)
    common = {"Wv": Wv16, "bv_b": bv_b, "ident": ident}
    if fuse_scores:
        G = np.asarray(Wq, np.float64) @ np.asarray(Wk, np.float64).T
        common["Gt"] = np.ascontiguousarray(G.T, f16)
    else:
        common["Wq"] = np.ascontiguousarray(Wq, f16)
        common["Wk"] = np.ascontiguousarray(Wk, f16)
        common["bq_pk"] = np.asarray(bq, f32).reshape(HC, 128).T.copy()
        common["bk_pk"] = np.asarray(bk, f32).reshape(HC, 128).T.copy()
    in_maps = []
    for c in range(n_cores):
        lo, hi = c * b_per_core, (c + 1) * b_per_core
        in_maps.append({
            "x_to": np.ascontiguousarray(x_to[lo:hi]),
            "x_from": np.ascontiguousarray(x_from[lo:hi]),
            **common,
        })
    return in_maps


_NC_CACHE = {}


def run(x_to, x_from, Wq, bq, Wk, bk, Wv, bv, trace=False, trace_kwargs=None,
        tmpdir=None):
    from concourse.bass_utils import run_bass_kernel_spmd

    B, S, D = np.asarray(x_to).shape
    N_CORES = 8
    assert B % N_CORES == 0
    BPC = B // N_CORES

    fuse = bool(np.all(np.asarray(bq) == 0) and np.all(np.asarray(bk) == 0))
    key = (BPC, S, D, fuse)
    if key not in _NC_CACHE:
        _NC_CACHE[key] = build_attention_nc(BPC, S, D, fuse_scores=fuse)
    nc = _NC_CACHE[key]

    in_maps = _host_inputs(x_to, x_from, Wq, bq, Wk, bk, Wv, bv, N_CORES, BPC, D,
                           fuse)
    res = run_bass_kernel_spmd(
        nc, in_maps, list(range(N_CORES)), trace=trace,
        trace_kwargs=trace_kwargs or {}, tmpdir=tmpdir,
    )
    outp = np.concatenate([res.results[i]["out"] for i in range(N_CORES)], axis=0)
    return outp, res


def kernel(x_to, x_from, Wq, bq, Wk, bk, Wv, bv):
    outp, _ = run(x_to, x_from, Wq, bq, Wk, bk, Wv, bv)
    return outp


# revision 26
# speedup vs baseline: 1.2836x; 1.0475x over previous
"""Bass/Tile attention kernel for trn2, data-parallel over batch on 8 cores.

Computes, per batch b:
    q = x_to @ Wq + bq ; k = x_from @ Wk + bk ; v = x_from @ Wv + bv
    out = softmax(q k^T / sqrt(H)) @ v

Per-core layout strategy (2 batches per core):
  - All matmul operands fp16 (x and W rounded on host; fp32 PSUM
    accumulation).  Modeled end-to-end error vs the fp32 reference is
    ~3.5e-4 of the output absmax — the softmax averaging washes out
    elementwise rounding.
  - x transposed on PE (identity matmul, fp16 fast-weight-load) into
    xT [d, seq] tiles.
  - Scores fused: scores = x_to (Wq Wk^T) x_from^T with G = Wq Wk^T
    precomputed on host (0.14% of total FLOPs), so only ONE projection
    (uT = G x_from^T) is needed instead of two.  Valid when bq = bk = 0
    (true for this problem); otherwise falls back to separate q/k
    projections.
  - Scores computed TRANSPOSED: sT[k, q] = uT_chunk^T @ x_toT, so the
    exp'd scores feed the second matmul as lhsT with no transposes.
    Softmax denominator comes free from a ones-column appended to v
    (column D of the attn output accumulates the exp sum).  No max
    subtraction (scores are O(1) at this problem's scale).
  - Software-pipelined: the next q-block's transposes run in the middle
    of the current block's attn matmuls, and a dummy-matmul warmup keeps
    the PE HAM clock gate at 8/8 from the first real matmul on.
"""

import sys

sys.path.insert(0, "/opt/trn_rl_repo")

import numpy as np

import concourse.bacc as bacc
import concourse.mybir as mybir
import concourse.tile as tile

F32 = mybir.dt.float32
FP16 = mybir.dt.float16


def build_attention_nc(B_PER_CORE, S, D, QB=512, fuse_scores=True):
    """Build the per-core Bass kernel. S = seq len, D = model dim = head dim."""
    assert D % 128 == 0 and S % 512 == 0 and QB % 128 == 0 and S % QB == 0
    HC = D // 128          # chunks of the model/head dim
    KC = S // 128          # 128-row chunks of the key sequence
    KBLK = S // 512        # 512-row key blocks (phase P granularity)
    NQB = S // QB          # q blocks
    QT_PER_B = QB // 128   # 128-row q tiles per q block
    NCHUNK = QB // 128     # x_to chunks per q block
    SCALE = float(1.0 / np.sqrt(np.float32(D)))

    nc = bacc.Bacc("TRN2", target_bir_lowering=False, debug=False)

    x_to = nc.declare_dram_parameter("x_to", [B_PER_CORE, S, D], FP16, isOutput=False).ap()
    x_from = nc.declare_dram_parameter("x_from", [B_PER_CORE, S, D], FP16, isOutput=False).ap()
    if fuse_scores:
        # Gt = (Wq @ Wk^T)^T, host-precomputed
        gt = nc.declare_dram_parameter("Gt", [D, D], FP16, isOutput=False).ap()
    else:
        wq = nc.declare_dram_parameter("Wq", [D, D], FP16, isOutput=False).ap()
        wk = nc.declare_dram_parameter("Wk", [D, D], FP16, isOutput=False).ap()
        bq_pk = nc.declare_dram_parameter("bq_pk", [128, HC], F32, isOutput=False).ap()
        bk_pk = nc.declare_dram_parameter("bk_pk", [128, HC], F32, isOutput=False).ap()
    wv = nc.declare_dram_parameter("Wv", [D, D], FP16, isOutput=False).ap()
    bv_b = nc.declare_dram_parameter("bv_b", [128, D + 2], F32, isOutput=False).ap()
    out = nc.declare_dram_parameter("out", [B_PER_CORE, S, D], F32, isOutput=True).ap()

    with tile.TileContext(nc) as tc:
        import contextlib

        with contextlib.ExitStack() as ctx:
            const = ctx.enter_context(tc.tile_pool(name="const", bufs=1))
            work = ctx.enter_context(tc.tile_pool(name="work", bufs=1))
            psum = ctx.enter_context(tc.tile_pool(name="psum", bufs=1, space="PSUM"))

            # ---- constants (small, front of the DMA queues) ----
            if not fuse_scores:
                bq_sb = const.tile([128, HC], F32, name="bq_sb")
                nc.sync.dma_start(out=bq_sb[:], in_=bq_pk[:])
                bk_sb = const.tile([128, HC], F32, name="bk_sb")
                nc.sync.dma_start(out=bk_sb[:], in_=bk_pk[:])
            # PE warm-up: dummy matmuls on a zeroed tile so the HAM clock
            # gate reaches 8/8 before the first real matmul, sized to also
            # cover the weight-DMA arrival (~17us).
            warm = const.tile([128, 128], FP16, name="warm")
            nc.gpsimd.memset(warm[:], 0.0)
            pw = psum.tile([128, 128], F32, name="ps_a", bufs=4)
            wg_sb, wv_sb, wq_sb = [], [], []

            def load_weights():
                # one big DMA per weight matrix: [D, D] -> [128, HC*D] with
                # chunk c at columns [c*D, (c+1)*D)
                wv_all = const.tile([128, HC, D], FP16, name="wv_all")
                nc.gpsimd.dma_start(
                    out=wv_all[:], in_=wv.rearrange("(c p) h -> p c h", p=128))
                wv_sb.extend(wv_all[:, d, :] for d in range(HC))
                if fuse_scores:
                    wg_all = const.tile([128, HC, D], FP16, name="wg_all")
                    nc.gpsimd.dma_start(
                        out=wg_all[:], in_=gt.rearrange("(c p) h -> p c h", p=128))
                    wg_sb.extend(wg_all[:, d, :] for d in range(HC))
                else:
                    wk_all = const.tile([128, HC, D], FP16, name="wk_all")
                    nc.gpsimd.dma_start(
                        out=wk_all[:], in_=wk.rearrange("(c p) h -> p c h", p=128))
                    wg_sb.extend(wk_all[:, d, :] for d in range(HC))
                    wq_all = const.tile([128, HC, D], FP16, name="wq_all")
                    nc.gpsimd.dma_start(
                        out=wq_all[:], in_=wq.rearrange("(c p) h -> p c h", p=128))
                    wq_sb.extend(wq_all[:, d, :] for d in range(HC))

            load_weights()
            for i in range(96):
                nc.tensor.matmul(pw[:], warm[:], warm[:],
                                 start=(i == 0), stop=(i == 95))
            bvb_sb = const.tile([128, D + 2], F32, name="bvb_sb")
            nc.gpsimd.dma_start(out=bvb_sb[:], in_=bv_b[:])

            # free-dim splits for matmul outputs (PSUM bank = 512 f32).
            d_splits = [(i, min(512, D - i)) for i in range(0, D, 512)]
            o_splits = [(i, min(512, D + 2 - i)) for i in range(0, D + 2, 512)]

            for b in range(B_PER_CORE):
                # uT = G @ x_from^T (fused) or kT = Wk^T x_from^T (fallback):
                # either way the scores lhsT, [D, S] in HC tiles.
                uT = [work.tile([128, S], FP16, name="uT", bufs=HC + 1)
                      for _ in range(HC)]
                vts = []

                # -- prepare a q block's x_to^T tiles via DMA-xbar
                #    transpose (fp16: 2-byte dtype makes this legal) --
                def prep_q(qb, xqT):
                    q0 = qb * QB
                    for d in range(HC):
                        nc.sync.dma_start(
                            out=xqT[d][:],
                            in_=x_to[b, q0:q0 + QB, d * 128:(d + 1) * 128],
                            transpose=True)

                def new_xqT():
                    return [work.tile([128, QB], FP16, name="xqT", bufs=HC + 4)
                            for _ in range(HC)]

                def proj_q(xqT):
                    """Unfused fallback: qT = Wq^T x_to^T + bq."""
                    qT = [work.tile([128, QB], FP16, name="qT", bufs=2 * HC)
                          for _ in range(HC)]
                    for h in range(HC):
                        pq = psum.tile([128, QB], F32, name="ps_a", bufs=4)
                        for d in range(HC):
                            nc.tensor.matmul(
                                pq[:],
                                wq_sb[d][:, h * 128:(h + 1) * 128],
                                xqT[d][:],
                                start=(d == 0), stop=(d == HC - 1),
                            )
                        nc.scalar.activation(
                            out=qT[h][:], in_=pq[:],
                            func=mybir.ActivationFunctionType.Identity,
                            bias=bq_sb[:, h:h + 1],
                        )
                    return qT

                # ======== Phase P: x_from -> uT (or kT), v_ext ========
                xqT = None
                for kb in range(KBLK):
                    if kb == KBLK - 1:
                        xqT = new_xqT()
                    xfT = [work.tile([128, 512], FP16, name="xT", bufs=2 * HC)
                           for _ in range(HC)]
                    r0 = kb * 512
                    for d in range(HC):
                        nc.sync.dma_start(
                            out=xfT[d][:],
                            in_=x_from[b, r0:r0 + 512, d * 128:(d + 1) * 128],
                            transpose=True)
                    for j in range(4):
                        # v projection for this 128-row chunk
                        pv = psum.tile([128, D + 2], F32, name="ps_o", bufs=2)
                        for (c0, cw) in d_splits:
                            for d in range(HC):
                                nc.tensor.matmul(
                                    pv[:, c0:c0 + cw],
                                    xfT[d][:, j * 128:(j + 1) * 128],
                                    wv_sb[d][:, c0:c0 + cw],
                                    start=(d == 0), stop=(d == HC - 1),
                                )
                        vt = work.tile([128, D + 2], FP16, name="v", bufs=KC + 4)
                        nc.vector.tensor_add(vt[:, :D], pv[:, :D], bvb_sb[:, :D])
                        nc.vector.tensor_copy(out=vt[:, D:D + 2], in_=bvb_sb[:, D:D + 2])
                        vts.append(vt)
                        # issue block-0 q loads during the last phase-P block
                        if kb == KBLK - 1 and j == 1:
                            prep_q(0, xqT)
                        if j % 2 == 1:
                            # uT/kT projection for the finished half-block
                            c0 = kb * 512 + (j - 1) * 128
                            for h in range(HC):
                                pk = psum.tile([128, 256], F32, name="ps_a", bufs=4)
                                for d in range(HC):
                                    nc.tensor.matmul(
                                        pk[:],
                                        wg_sb[d][:, h * 128:(h + 1) * 128],
                                        xfT[d][:, (j - 1) * 128:(j + 1) * 128],
                                        start=(d == 0), stop=(d == HC - 1),
                                    )
                                if fuse_scores:
                                    if h % 2 == 0:
                                        nc.scalar.copy(out=uT[h][:, c0:c0 + 256], in_=pk[:])
                                    else:
                                        nc.vector.tensor_copy(out=uT[h][:, c0:c0 + 256], in_=pk[:])
                                else:
                                    nc.scalar.activation(
                                        out=uT[h][:, c0:c0 + 256], in_=pk[:],
                                        func=mybir.ActivationFunctionType.Identity,
                                        bias=bk_sb[:, h:h + 1],
                                    )

                # ======== Phase A: q blocks (software-pipelined) ========
                sc_rhs = xqT if fuse_scores else proj_q(xqT)

                for qb in range(NQB):
                    q0 = qb * QB
                    # transposed scores + fused scale/exp eviction
                    ex = [work.tile([128, QB], FP16, name="expT", bufs=KC + 4)
                          for _ in range(KC)]
                    for kc in range(KC):
                        ps = psum.tile([128, QB], F32, name="ps_a", bufs=4)
                        for h in range(HC):
                            nc.tensor.matmul(
                                ps[:],
                                uT[h][:, kc * 128:(kc + 1) * 128],
                                sc_rhs[h][:],
                                start=(h == 0), stop=(h == HC - 1),
                            )
                        nc.scalar.activation(
                            out=ex[kc][:], in_=ps[:],
                            func=mybir.ActivationFunctionType.Exp,
                            scale=SCALE,
                        )
                    # attn @ v_ext (+ denominator column), interleaved with
                    # the next block's x_to transposes; normalize, store
                    if qb + 1 < NQB:
                        xqT = new_xqT()
                    for t in range(QT_PER_B):
                        po = psum.tile([128, D + 2], F32, name="ps_o", bufs=2)
                        for kc in range(KC):
                            for (c0, cw) in o_splits:
                                nc.tensor.matmul(
                                    po[:, c0:c0 + cw],
                                    ex[kc][:, t * 128:(t + 1) * 128],
                                    vts[kc][:, c0:c0 + cw],
                                    start=(kc == 0), stop=(kc == KC - 1),
                                )
                        if qb + 1 < NQB and t == 0:
                            prep_q(qb + 1, xqT)
                        rec = work.tile([128, 1], F32, name="rec", bufs=4)
                        nc.vector.reciprocal(rec[:], po[:, D:D + 1])
                        ot = work.tile([128, D], F32, name="ot", bufs=3)
                        nc.vector.tensor_scalar_mul(ot[:], po[:, :D], rec[:])
                        row0 = q0 + t * 128
                        nc.sync.dma_start(out=out[b, row0:row0 + 128, :], in_=ot[:])
                    if qb + 1 < NQB:
                        sc_rhs = xqT if fuse_scores else proj_q(xqT)

    nc.compile()
    return nc


def _host_inputs(x_to, x_from, Wq, bq, Wk, bk, Wv, bv, n_cores, b_per_core, D,
                 fuse_scores):
    HC = D // 128
    f32, f16 = np.float32, np.float16
    bv_ext = np.concatenate([np.asarray(bv, f32), np.array([1.0, 0.0], f32)])
    bv_b = np.tile(bv_ext[None, :], (128, 1)).copy()
    Wv16 = np.ascontiguousarray(Wv, f16)
    x_to = np.asarray(x_to, f16)
    x_from = np.asarray(x_from, f16)
    common = {"Wv": Wv16, "bv_b": bv_b}
    if fuse_scores:
        G = np.asarray(Wq, np.float64) @ np.asarray(Wk, np.float64).T
        common["Gt"] = np.ascontiguousarray(G.T, f16)
    else:
        common["Wq"] = np.ascontiguousarray(Wq, f16)
        common["Wk"] = np.ascontiguousarray(Wk, f16)
        common["bq_pk"] = np.asarray(bq, f32).reshape(HC, 128).T.copy()
        common["bk_pk"] = np.asarray(bk, f32).reshape(HC, 128).T.copy()
    in_maps = []
    for c in range(n_cores):
        lo, hi = c * b_per_core, (c + 1) * b_per_core
        in_maps.append({
            "x_to": np.ascontiguousarray(x_to[lo:hi]),
            "x_from": np.ascontiguousarray(x_from[lo:hi]),
            **common,
        })
    return in_maps


_NC_CACHE = {}


def run(x_to, x_from, Wq, bq, Wk, bk, Wv, bv, trace=False, trace_kwargs=None,
        tmpdir=None):
    from concourse.bass_utils import run_bass_kernel_spmd

    B, S, D = np.asarray(x_to).shape
    N_CORES = 8
    assert B % N_CORES == 0
    BPC = B // N_CORES

    fuse = bool(np.all(np.asarray(bq) == 0) and np.all(np.asarray(bk) == 0))
    key = (BPC, S, D, fuse)
    if key not in _NC_CACHE:
        _NC_CACHE[key] = build_attention_nc(BPC, S, D, fuse_scores=fuse)
    nc = _NC_CACHE[key]

    in_maps = _host_inputs(x_to, x_from, Wq, bq, Wk, bk, Wv, bv, N_CORES, BPC, D,
                           fuse)
    res = run_bass_kernel_spmd(
        nc, in_maps, list(range(N_CORES)), trace=trace,
        trace_kwargs=trace_kwargs or {}, tmpdir=tmpdir,
    )
    outp = np.concatenate([res.results[i]["out"] for i in range(N_CORES)], axis=0)
    return outp, res


def kernel(x_to, x_from, Wq, bq, Wk, bk, Wv, bv):
    outp, _ = run(x_to, x_from, Wq, bq, Wk, bk, Wv, bv)
    return outp


# revision 27
# speedup vs baseline: 1.2884x; 1.0037x over previous
"""Bass/Tile attention kernel for trn2, data-parallel over batch on 8 cores.

Computes, per batch b:
    q = x_to @ Wq + bq ; k = x_from @ Wk + bk ; v = x_from @ Wv + bv
    out = softmax(q k^T / sqrt(H)) @ v

Per-core layout strategy (2 batches per core):
  - All matmul operands fp16 (x and W rounded on host; fp32 PSUM
    accumulation).  Modeled end-to-end error vs the fp32 reference is
    ~3.5e-4 of the output absmax — the softmax averaging washes out
    elementwise rounding.
  - x transposed on PE (identity matmul, fp16 fast-weight-load) into
    xT [d, seq] tiles.
  - Scores fused: scores = x_to (Wq Wk^T) x_from^T with G = Wq Wk^T
    precomputed on host (0.14% of total FLOPs), so only ONE projection
    (uT = G x_from^T) is needed instead of two.  Valid when bq = bk = 0
    (true for this problem); otherwise falls back to separate q/k
    projections.
  - Scores computed TRANSPOSED: sT[k, q] = uT_chunk^T @ x_toT, so the
    exp'd scores feed the second matmul as lhsT with no transposes.
    Softmax denominator comes free from a ones-column appended to v
    (column D of the attn output accumulates the exp sum).  No max
    subtraction (scores are O(1) at this problem's scale).
  - Software-pipelined: the next q-block's transposes run in the middle
    of the current block's attn matmuls, and a dummy-matmul warmup keeps
    the PE HAM clock gate at 8/8 from the first real matmul on.
"""

import sys

sys.path.insert(0, "/opt/trn_rl_repo")

import numpy as np

import concourse.bacc as bacc
import concourse.mybir as mybir
import concourse.tile as tile

F32 = mybir.dt.float32
FP16 = mybir.dt.float16


def build_attention_nc(B_PER_CORE, S, D, QB=512, fuse_scores=True):
    """Build the per-core Bass kernel. S = seq len, D = model dim = head dim."""
    assert D % 128 == 0 and S % 512 == 0 and QB % 128 == 0 and S % QB == 0
    HC = D // 128          # chunks of the model/head dim
    KC = S // 128          # 128-row chunks of the key sequence
    KBLK = S // 512        # 512-row key blocks (phase P granularity)
    NQB = S // QB          # q blocks
    QT_PER_B = QB // 128   # 128-row q tiles per q block
    NCHUNK = QB // 128     # x_to chunks per q block
    SCALE = float(1.0 / np.sqrt(np.float32(D)))

    nc = bacc.Bacc("TRN2", target_bir_lowering=False, debug=False)

    x_to = nc.declare_dram_parameter("x_to", [B_PER_CORE, S, D], FP16, isOutput=False).ap()
    x_from = nc.declare_dram_parameter("x_from", [B_PER_CORE, S, D], FP16, isOutput=False).ap()
    if fuse_scores:
        # Gt = (Wq @ Wk^T)^T, host-precomputed
        gt = nc.declare_dram_parameter("Gt", [D, D], FP16, isOutput=False).ap()
    else:
        wq = nc.declare_dram_parameter("Wq", [D, D], FP16, isOutput=False).ap()
        wk = nc.declare_dram_parameter("Wk", [D, D], FP16, isOutput=False).ap()
        bq_pk = nc.declare_dram_parameter("bq_pk", [128, HC], F32, isOutput=False).ap()
        bk_pk = nc.declare_dram_parameter("bk_pk", [128, HC], F32, isOutput=False).ap()
    wv = nc.declare_dram_parameter("Wv", [D, D], FP16, isOutput=False).ap()
    bv_b = nc.declare_dram_parameter("bv_b", [128, D + 2], F32, isOutput=False).ap()
    out = nc.declare_dram_parameter("out", [B_PER_CORE, S, D], F32, isOutput=True).ap()

    with tile.TileContext(nc) as tc:
        import contextlib

        with contextlib.ExitStack() as ctx:
            const = ctx.enter_context(tc.tile_pool(name="const", bufs=1))
            work = ctx.enter_context(tc.tile_pool(name="work", bufs=1))
            psum = ctx.enter_context(tc.tile_pool(name="psum", bufs=1, space="PSUM"))

            # ---- constants (small, front of the DMA queues) ----
            if not fuse_scores:
                bq_sb = const.tile([128, HC], F32, name="bq_sb")
                nc.sync.dma_start(out=bq_sb[:], in_=bq_pk[:])
                bk_sb = const.tile([128, HC], F32, name="bk_sb")
                nc.sync.dma_start(out=bk_sb[:], in_=bk_pk[:])
            # PE warm-up: dummy matmuls on a zeroed tile so the HAM clock
            # gate reaches 8/8 before the first real matmul, sized to also
            # cover the weight-DMA arrival (~17us).
            warm = const.tile([128, 128], FP16, name="warm")
            nc.gpsimd.memset(warm[:], 0.0)
            pw = psum.tile([128, 128], F32, name="ps_a", bufs=4)
            wg_sb, wv_sb, wq_sb = [], [], []

            def load_weights():
                # one big DMA per weight matrix: [D, D] -> [128, HC*D] with
                # chunk c at columns [c*D, (c+1)*D)
                wv_all = const.tile([128, HC, D], FP16, name="wv_all")
                nc.gpsimd.dma_start(
                    out=wv_all[:], in_=wv.rearrange("(c p) h -> p c h", p=128))
                wv_sb.extend(wv_all[:, d, :] for d in range(HC))
                if fuse_scores:
                    wg_all = const.tile([128, HC, D], FP16, name="wg_all")
                    nc.gpsimd.dma_start(
                        out=wg_all[:], in_=gt.rearrange("(c p) h -> p c h", p=128))
                    wg_sb.extend(wg_all[:, d, :] for d in range(HC))
                else:
                    wk_all = const.tile([128, HC, D], FP16, name="wk_all")
                    nc.gpsimd.dma_start(
                        out=wk_all[:], in_=wk.rearrange("(c p) h -> p c h", p=128))
                    wg_sb.extend(wk_all[:, d, :] for d in range(HC))
                    wq_all = const.tile([128, HC, D], FP16, name="wq_all")
                    nc.gpsimd.dma_start(
                        out=wq_all[:], in_=wq.rearrange("(c p) h -> p c h", p=128))
                    wq_sb.extend(wq_all[:, d, :] for d in range(HC))

            load_weights()
            for i in range(176):
                nc.tensor.matmul(pw[:], warm[:], warm[:],
                                 start=(i == 0), stop=(i == 175))
            bvb_sb = const.tile([128, D + 2], F32, name="bvb_sb")
            nc.gpsimd.dma_start(out=bvb_sb[:], in_=bv_b[:])

            # free-dim splits for matmul outputs (PSUM bank = 512 f32).
            d_splits = [(i, min(512, D - i)) for i in range(0, D, 512)]
            o_splits = [(i, min(512, D + 2 - i)) for i in range(0, D + 2, 512)]

            for b in range(B_PER_CORE):
                # uT = G @ x_from^T (fused) or kT = Wk^T x_from^T (fallback):
                # either way the scores lhsT, [D, S] in HC tiles.
                uT = [work.tile([128, S], FP16, name="uT", bufs=HC + 1)
                      for _ in range(HC)]
                vts = []

                # -- prepare a q block's x_to^T tiles via DMA-xbar
                #    transpose (fp16: 2-byte dtype makes this legal) --
                def prep_q(qb, xqT):
                    q0 = qb * QB
                    for d in range(HC):
                        nc.sync.dma_start(
                            out=xqT[d][:],
                            in_=x_to[b, q0:q0 + QB, d * 128:(d + 1) * 128],
                            transpose=True)

                def new_xqT():
                    return [work.tile([128, QB], FP16, name="xqT", bufs=2 * HC)
                            for _ in range(HC)]

                def proj_q(xqT):
                    """Unfused fallback: qT = Wq^T x_to^T + bq."""
                    qT = [work.tile([128, QB], FP16, name="qT", bufs=2 * HC)
                          for _ in range(HC)]
                    for h in range(HC):
                        pq = psum.tile([128, QB], F32, name="ps_a", bufs=4)
                        for d in range(HC):
                            nc.tensor.matmul(
                                pq[:],
                                wq_sb[d][:, h * 128:(h + 1) * 128],
                                xqT[d][:],
                                start=(d == 0), stop=(d == HC - 1),
                            )
                        nc.scalar.activation(
                            out=qT[h][:], in_=pq[:],
                            func=mybir.ActivationFunctionType.Identity,
                            bias=bq_sb[:, h:h + 1],
                        )
                    return qT

                # ======== Phase P: x_from -> uT (or kT), v_ext ========
                xqT = None
                for kb in range(KBLK):
                    if kb == KBLK - 1:
                        xqT = new_xqT()
                    xfT = [work.tile([128, 512], FP16, name="xT", bufs=4 * HC)
                           for _ in range(HC)]
                    r0 = kb * 512
                    for d in range(HC):
                        nc.sync.dma_start(
                            out=xfT[d][:],
                            in_=x_from[b, r0:r0 + 512, d * 128:(d + 1) * 128],
                            transpose=True)
                    for j in range(4):
                        # v projection for this 128-row chunk
                        pv = psum.tile([128, D + 2], F32, name="ps_o", bufs=2)
                        for (c0, cw) in d_splits:
                            for d in range(HC):
                                nc.tensor.matmul(
                                    pv[:, c0:c0 + cw],
                                    xfT[d][:, j * 128:(j + 1) * 128],
                                    wv_sb[d][:, c0:c0 + cw],
                                    start=(d == 0), stop=(d == HC - 1),
                                )
                        vt = work.tile([128, D + 2], FP16, name="v", bufs=KC + 4)
                        nc.vector.tensor_add(vt[:, :D], pv[:, :D], bvb_sb[:, :D])
                        nc.vector.tensor_copy(out=vt[:, D:D + 2], in_=bvb_sb[:, D:D + 2])
                        vts.append(vt)
                        # issue block-0 q loads during the last phase-P block
                        if kb == KBLK - 1 and j == 1:
                            prep_q(0, xqT)
                        if j % 2 == 1:
                            # uT/kT projection for the finished half-block
                            c0 = kb * 512 + (j - 1) * 128
                            for h in range(HC):
                                pk = psum.tile([128, 256], F32, name="ps_a", bufs=4)
                                for d in range(HC):
                                    nc.tensor.matmul(
                                        pk[:],
                                        wg_sb[d][:, h * 128:(h + 1) * 128],
                                        xfT[d][:, (j - 1) * 128:(j + 1) * 128],
                                        start=(d == 0), stop=(d == HC - 1),
                                    )
                                if fuse_scores:
                                    if h % 2 == 0:
                                        nc.scalar.copy(out=uT[h][:, c0:c0 + 256], in_=pk[:])
                                    else:
                                        nc.vector.tensor_copy(out=uT[h][:, c0:c0 + 256], in_=pk[:])
                                else:
                                    nc.scalar.activation(
                                        out=uT[h][:, c0:c0 + 256], in_=pk[:],
                                        func=mybir.ActivationFunctionType.Identity,
                                        bias=bk_sb[:, h:h + 1],
                                    )

                # ======== Phase A: q blocks (software-pipelined) ========
                sc_rhs = xqT if fuse_scores else proj_q(xqT)

                for qb in range(NQB):
                    q0 = qb * QB
                    # transposed scores + fused scale/exp eviction
                    ex = [work.tile([128, QB], FP16, name="expT", bufs=KC + 4)
                          for _ in range(KC)]
                    for kc in range(KC):
                        ps = psum.tile([128, QB], F32, name="ps_a", bufs=4)
                        for h in range(HC):
                            nc.tensor.matmul(
                                ps[:],
                                uT[h][:, kc * 128:(kc + 1) * 128],
                                sc_rhs[h][:],
                                start=(h == 0), stop=(h == HC - 1),
                            )
                        nc.scalar.activation(
                            out=ex[kc][:], in_=ps[:],
                            func=mybir.ActivationFunctionType.Exp,
                            scale=SCALE,
                        )
                    # attn @ v_ext (+ denominator column), interleaved with
                    # the next block's x_to transposes; normalize, store
                    if qb + 1 < NQB:
                        xqT = new_xqT()
                    for t in range(QT_PER_B):
                        po = psum.tile([128, D + 2], F32, name="ps_o", bufs=2)
                        for kc in range(KC):
                            for (c0, cw) in o_splits:
                                nc.tensor.matmul(
                                    po[:, c0:c0 + cw],
                                    ex[kc][:, t * 128:(t + 1) * 128],
                                    vts[kc][:, c0:c0 + cw],
                                    start=(kc == 0), stop=(kc == KC - 1),
                                )
                        if qb + 1 < NQB and t == 0:
                            prep_q(qb + 1, xqT)
                        rec = work.tile([128, 1], F32, name="rec", bufs=4)
                        nc.vector.reciprocal(rec[:], po[:, D:D + 1])
                        ot = work.tile([128, D], F32, name="ot", bufs=3)
                        nc.vector.tensor_scalar_mul(ot[:], po[:, :D], rec[:])
                        row0 = q0 + t * 128
                        nc.sync.dma_start(out=out[b, row0:row0 + 128, :], in_=ot[:])
                    if qb + 1 < NQB:
                        sc_rhs = xqT if fuse_scores else proj_q(xqT)

    nc.compile()
    return nc


def _host_inputs(x_to, x_from, Wq, bq, Wk, bk, Wv, bv, n_cores, b_per_core, D,
                 fuse_scores):
    HC = D // 128
    f32, f16 = np.float32, np.float16
    bv_ext = np.concatenate([np.asarray(bv, f32), np.array([1.0, 0.0], f32)])
    bv_b = np.tile(bv_ext[None, :], (128, 1)).copy()
    Wv16 = np.ascontiguousarray(Wv, f16)
    x_to = np.asarray(x_to, f16)
    x_from = np.asarray(x_from, f16)
    common = {"Wv": Wv16, "bv_b": bv_b}
    if fuse_scores:
        G = np.asarray(Wq, np.float64) @ np.asarray(Wk, np.float64).T
        common["Gt"] = np.ascontiguousarray(G.T, f16)
    else:
        common["Wq"] = np.ascontiguousarray(Wq, f16)
        common["Wk"] = np.ascontiguousarray(Wk, f16)
        common["bq_pk"] = np.asarray(bq, f32).reshape(HC, 128).T.copy()
        common["bk_pk"] = np.asarray(bk, f32).reshape(HC, 128).T.copy()
    in_maps = []
    for c in range(n_cores):
        lo, hi = c * b_per_core, (c + 1) * b_per_core
        in_maps.append({
            "x_to": np.ascontiguousarray(x_to[lo:hi]),
            "x_from": np.ascontiguousarray(x_from[lo:hi]),
            **common,
        })
    return in_maps


_NC_CACHE = {}


def run(x_to, x_from, Wq, bq, Wk, bk, Wv, bv, trace=False, trace_kwargs=None,
        tmpdir=None):
    from concourse.bass_utils import run_bass_kernel_spmd

    B, S, D = np.asarray(x_to).shape
    N_CORES = 8
    assert B % N_CORES == 0
    BPC = B // N_CORES

    fuse = bool(np.all(np.asarray(bq) == 0) and np.all(np.asarray(bk) == 0))
    key = (BPC, S, D, fuse)
    if key not in _NC_CACHE:
        _NC_CACHE[key] = build_attention_nc(BPC, S, D, fuse_scores=fuse)
    nc = _NC_CACHE[key]

    in_maps = _host_inputs(x_to, x_from, Wq, bq, Wk, bk, Wv, bv, N_CORES, BPC, D,
                           fuse)
    res = run_bass_kernel_spmd(
        nc, in_maps, list(range(N_CORES)), trace=trace,
        trace_kwargs=trace_kwargs or {}, tmpdir=tmpdir,
    )
    outp = np.concatenate([res.results[i]["out"] for i in range(N_CORES)], axis=0)
    return outp, res


def kernel(x_to, x_from, Wq, bq, Wk, bk, Wv, bv):
    outp, _ = run(x_to, x_from, Wq, bq, Wk, bk, Wv, bv)
    return outp
